# revision 1
# baseline (speedup 1.0000x reference)
"""Causal self-attention kernel for 8 trn2 NeuronCores.

Sharding: core c = (b, g) with b = c // 2 (batch), g = c % 2 (head-group of 8
heads, Megatron column split of Wq/Wk/Wv + row split of Wp). Each core computes
a partial Y for its batch; host sums the two partials per batch.

Per-core dataflow (all matmuls in float32r = full-rate TF32-ish):
  - host pre-transposes x[b] -> xT [C, T] so contraction dims land on
    partitions without any on-device transposes.
  - QKV projections produce Q^T, K^T in [head-pair (128), T] layout and V in
    [T, headcols] layout (with a ones-column per head for free softmax sums).
  - scores are computed transposed: S^T = K @ Q^T per (pair, ktile) with
    row-tiled head pairs (K=64 contraction x2 concurrent).
  - softmax: exp on ACT straight out of PSUM (scale=1/8 folded in), causal
    mask via gpsimd affine_select on the exp output, colsum rides as row 64
    of the PV matmul (ones-augmented V, M=65).
  - PV: O^T[h] = V_aug[h]^T @ P^T accumulated over ktiles in PSUM.
  - normalization: recip(colsum) broadcast down 64 partitions with a K=1
    ones matmul, multiplied during PSUM->SBUF eviction.
  - out-proj: Y += A^T-tiles.T @ Wp rows, row-tiled head pairs, + bias.
"""

import numpy as np

import concourse.bacc as bacc
import concourse.bass as bass
import concourse.mybir as mybir
import concourse.tile as tile

F32 = mybir.dt.float32
F32R = mybir.dt.float32r
AF = mybir.ActivationFunctionType
ALU = mybir.AluOpType

import os as _os
B, T, C = 4, int(_os.environ.get("KT_T", "2048")), 1024
H, D = 16, 64
G = 2  # head-group shards (cores per batch)
GC = C // G  # 512 output cols per shard
P = 128
NCT = C // P  # 8 contraction tiles over C
TCH = 512  # T chunk (= PSUM bank in fp32)
NTCH = T // TCH  # 4
NPAIR = GC // P  # 4 head pairs per core
NKT = T // P  # 16 key tiles
HPC = H // G  # 8 heads per core
VW = D + 1  # V columns per head incl. ones column


def bcast_ap(h, parts, free):
    """DRAM [free] vector -> [parts, free] partition-broadcast AP."""
    ap = h[:]
    return bass.AP(tensor=ap.tensor, offset=ap.offset, ap=[[0, parts], [1, free]])


def build_nc(stop_phase="C"):
    nc = bacc.Bacc("TRN2", target_bir_lowering=False)

    xT = nc.dram_tensor("xT", [C, T], F32R, kind="ExternalInput")
    wq = nc.dram_tensor("wq", [C, GC], F32R, kind="ExternalInput")
    wk = nc.dram_tensor("wk", [C, GC], F32R, kind="ExternalInput")
    wv = nc.dram_tensor("wv", [C, GC], F32R, kind="ExternalInput")
    wp = nc.dram_tensor("wp", [GC, C], F32R, kind="ExternalInput")
    bqd = nc.dram_tensor("bq", [GC], F32, kind="ExternalInput")
    bkd = nc.dram_tensor("bk", [GC], F32, kind="ExternalInput")
    bvd = nc.dram_tensor("bv", [GC], F32, kind="ExternalInput")
    bpd = nc.dram_tensor("bp", [C], F32, kind="ExternalInput")
    ones64 = nc.dram_tensor("ones64", [P, P], F32R, kind="ExternalInput")
    y = nc.dram_tensor(
        "y",
        [T, C],
        F32R if stop_phase in ("A", "B", "CR", "CD5") else F32,
        kind="ExternalOutput",
    )

    xT_r = xT[:, :].rearrange("(o p) t -> p o t", p=P)  # [128, 8, T]
    wq_r = wq[:, :].rearrange("(o p) m -> p o m", p=P)  # [128, 8, 512]
    wk_r = wk[:, :].rearrange("(o p) m -> p o m", p=P)
    wv_r = wv[:, :].rearrange("(o p) m -> p o m", p=P)
    wp_r = wp[:, :].rearrange("(o p) m -> p o m", p=P)  # [128, 4, 1024]
    y_r = y[:, :].rearrange("(n p) c -> n p c", p=P)  # [16, 128, 1024]

    with tile.TileContext(nc) as tc:
        with (
            tc.tile_pool(name="persist", bufs=1) as persist,
            tc.tile_pool(name="small", bufs=1) as small,
        ):
            # ---- persistent SBUF state ----
            qT_sb = persist.tile([P, NPAIR, T], F32R)  # 32KB/part
            kT_sb = persist.tile([P, NPAIR, T], F32R)  # 32KB/part
            v_sb = persist.tile([P, NKT, HPC, VW], F32R)  # ~33KB/part
            wp_sb = persist.tile([P, NPAIR, C], F32R)  # 16KB/part
            nc.sync.dma_start(out=wp_sb, in_=wp_r)

            bq_sb = small.tile([P, NPAIR], F32)
            nc.sync.dma_start(out=bq_sb, in_=bqd[:].rearrange("(o p) -> p o", p=P))
            bk_sb = small.tile([P, NPAIR], F32)
            nc.sync.dma_start(out=bk_sb, in_=bkd[:].rearrange("(o p) -> p o", p=P))
            bv_sb = small.tile([P, GC], F32)
            nc.gpsimd.dma_start(out=bv_sb, in_=bcast_ap(bvd, P, GC))
            bp_sb = small.tile([P, C], F32)
            nc.gpsimd.dma_start(out=bp_sb, in_=bcast_ap(bpd, P, C))
            ones_sb = small.tile([P, P], F32R)
            nc.sync.dma_start(out=ones_sb, in_=ones64[:, :])
            # ones column per head, copied from the ones tile
            nc.vector.tensor_copy(
                v_sb[:, :, :, D : D + 1],
                ones_sb[:, 0 : NKT * HPC].rearrange(
                    "p (a b o) -> p a b o", b=HPC, o=1
                ),
            )

            # ================= Phase A: QKV projections =================
            with (
                tc.tile_pool(name="wpool", bufs=1) as wpool,
                tc.tile_pool(name="xpool", bufs=2) as xpool,
                tc.tile_pool(name="prpsum", bufs=2, space="PSUM") as prpsum,
            ):
                wq_sb = wpool.tile([P, NCT, GC], F32R)
                nc.sync.dma_start(out=wq_sb, in_=wq_r)
                wk_sb = wpool.tile([P, NCT, GC], F32R)
                nc.sync.dma_start(out=wk_sb, in_=wk_r)
                wv_sb = wpool.tile([P, NCT, GC], F32R)
                nc.sync.dma_start(out=wv_sb, in_=wv_r)

                for tch in range(NTCH):
                    tsl = slice(tch * TCH, (tch + 1) * TCH)
                    xt = xpool.tile([P, NCT, TCH], F32R, name="xt")
                    nc.sync.dma_start(out=xt, in_=xT_r[:, :, tsl])
                    for pair in range(NPAIR):
                        psl = slice(pair * P, (pair + 1) * P)
                        q_ps = prpsum.tile([P, TCH], F32, name="q_ps")
                        for ct in range(NCT):
                            nc.tensor.matmul(
                                q_ps,
                                lhsT=wq_sb[:, ct, psl],
                                rhs=xt[:, ct, :],
                                start=(ct == 0),
                                stop=(ct == NCT - 1),
                            )
                        nc.vector.tensor_scalar_add(
                            qT_sb[:, pair, tsl], q_ps, bq_sb[:, pair : pair + 1]
                        )
                        k_ps = prpsum.tile([P, TCH], F32, name="k_ps")
                        for ct in range(NCT):
                            nc.tensor.matmul(
                                k_ps,
                                lhsT=wk_sb[:, ct, psl],
                                rhs=xt[:, ct, :],
                                start=(ct == 0),
                                stop=(ct == NCT - 1),
                            )
                        nc.vector.tensor_scalar_add(
                            kT_sb[:, pair, tsl], k_ps, bk_sb[:, pair : pair + 1]
                        )
                    for tloc in range(4):
                        tt = tch * 4 + tloc
                        v_ps = prpsum.tile([P, GC], F32, name="v_ps")
                        for ct in range(NCT):
                            nc.tensor.matmul(
                                v_ps,
                                lhsT=xt[:, ct, tloc * P : (tloc + 1) * P],
                                rhs=wv_sb[:, ct, :],
                                start=(ct == 0),
                                stop=(ct == NCT - 1),
                            )
                        nc.vector.tensor_tensor(
                            v_sb[:, tt, :, 0:D],
                            v_ps.rearrange("p (h d) -> p h d", d=D),
                            bv_sb.rearrange("p (h d) -> p h d", d=D),
                            ALU.add,
                        )

            if stop_phase == "A":
                for pair in range(NPAIR):
                    nc.sync.dma_start(out=y_r[pair, :, :], in_=qT_sb[:, pair, 0:1024])
                    nc.sync.dma_start(
                        out=y_r[4 + pair, :, :], in_=kT_sb[:, pair, 0:1024]
                    )
                for tt in range(4):
                    nc.sync.dma_start(
                        out=y_r[8 + tt, :, 0:GC].rearrange("p (h d) -> p h d", d=D),
                        in_=v_sb[:, tt, :, 0:D],
                    )

            # ================= Phase B + C: attention + out-proj =========
            if stop_phase != "A":
              with (
                  tc.tile_pool(name="stpsum", bufs=1, space="PSUM") as stpsum,
                  tc.tile_pool(name="opsum", bufs=1, space="PSUM") as opsum,
                  tc.tile_pool(name="bcpsum", bufs=1, space="PSUM") as bcpsum,
                  tc.tile_pool(name="ypsum", bufs=1, space="PSUM") as ypsum,
                  tc.tile_pool(name="ptpool", bufs=2) as ptpool,
                  tc.tile_pool(name="atpool", bufs=6) as atpool,
                  tc.tile_pool(name="mpool", bufs=2) as mpool,
                  tc.tile_pool(name="ypool", bufs=3) as ypool,
              ):
                  for qc in range(NTCH):
                      qsl = slice(qc * TCH, (qc + 1) * TCH)
                      nkt = (qc + 1) * 4
                      aTs = []
                      for pair in range(NPAIR):
                          o_ps_e = opsum.tile([D + 1, TCH], F32, name="o_ps_e")
                          o_ps_o = opsum.tile([D + 1, TCH], F32, name="o_ps_o")
                          gw = 1 if stop_phase == "C2" else 2
                          for g0 in range(0, nkt, gw):
                              st = stpsum.tile([P, gw, 2, TCH], F32, name="st")
                              for ti in range(gw):
                                  t = g0 + ti
                                  ksl = slice(t * P, (t + 1) * P)
                                  nc.tensor.matmul(
                                      st[:, ti, 0, :],
                                      lhsT=kT_sb[0:D, pair, ksl],
                                      rhs=qT_sb[0:D, pair, qsl],
                                      start=True,
                                      stop=True,
                                  )
                                  nc.tensor.matmul(
                                      st[:, ti, 1, :],
                                      lhsT=kT_sb[D:P, pair, ksl],
                                      rhs=qT_sb[D:P, pair, qsl],
                                      start=True,
                                      stop=True,
                                  )
                              pt = ptpool.tile([P, gw, 2, TCH], F32R, name="pt")
                              nc.scalar.activation(
                                  out=pt.rearrange("p a b n -> p (a b n)"),
                                  in_=st.rearrange("p a b n -> p (a b n)"),
                                  func=AF.Exp,
                                  scale=0.125,
                              )
                              for ti in range(gw):
                                  t = g0 + ti
                                  if t >= 4 * qc:  # diagonal block: causal mask
                                      for h in range(2):
                                          mv = pt[:, ti, h, :]
                                          nc.gpsimd.affine_select(
                                              out=mv,
                                              in_=mv,
                                              pattern=[[1, TCH]],
                                              compare_op=ALU.is_ge,
                                              fill=0.0,
                                              base=TCH * qc - P * t,
                                              channel_multiplier=-1,
                                          )
                              for ti in range(gw):
                                  t = g0 + ti
                                  nc.tensor.matmul(
                                      o_ps_e,
                                      lhsT=v_sb[:, t, 2 * pair, :],
                                      rhs=pt[:, ti, 0, :],
                                      start=(t == 0),
                                      stop=(t == nkt - 1),
                                      skip_group_check=True,
                                  )
                                  nc.tensor.matmul(
                                      o_ps_o,
                                      lhsT=v_sb[:, t, 2 * pair + 1, :],
                                      rhs=pt[:, ti, 1, :],
                                      start=(t == 0),
                                      stop=(t == nkt - 1),
                                      skip_group_check=True,
                                  )
                          # ---- normalize: aT = O^T * bcast(1/colsum) ----
                          recip = mpool.tile([P, 2, TCH], F32R, name="recip")
                          with nc.allow_low_precision(reason="softmax recip to f32r"):
                              nc.vector.reciprocal(
                                  recip[D : D + 1, 0, :], o_ps_e[D : D + 1, :]
                              )
                              nc.vector.reciprocal(
                                  recip[D : D + 1, 1, :], o_ps_o[D : D + 1, :]
                              )
                          aT = atpool.tile([P, TCH], F32R, name="aT")
                          bc_e = bcpsum.tile([D, TCH], F32, name="bc")
                          nc.tensor.matmul(
                              bc_e,
                              lhsT=ones_sb[D : D + 1, 0:D],
                              rhs=recip[D : D + 1, 0, :],
                              start=True,
                              stop=True,
                          )
                          bc_e_sb = mpool.tile([D, TCH], F32, name="bc_e_sb")
                          nc.vector.tensor_copy(bc_e_sb, bc_e)
                          nc.vector.tensor_tensor(
                              aT[0:D, :], o_ps_e[0:D, :], bc_e_sb, ALU.mult
                          )
                          bc_o = bcpsum.tile([D, TCH], F32, name="bc")
                          nc.tensor.matmul(
                              bc_o,
                              lhsT=ones_sb[D : D + 1, 0:D],
                              rhs=recip[D : D + 1, 1, :],
                              start=True,
                              stop=True,
                          )
                          bc_o_sb = mpool.tile([D, TCH], F32, name="bc_o_sb")
                          nc.vector.tensor_copy(bc_o_sb, bc_o)
                          stage = mpool.tile([D, TCH], F32R, name="stage")
                          nc.vector.tensor_tensor(stage, o_ps_o[0:D, :], bc_o_sb, ALU.mult)
                          nc.sync.dma_start(out=aT[D:P, :], in_=stage)
                          aTs.append(aT)
                          if stop_phase == "B":
                              nc.sync.dma_start(
                                  out=y_r[qc * 4 + pair, :, 0:TCH], in_=aT
                              )

                      if stop_phase == "B":
                          continue
                      # ---- out-proj for this q-chunk ----
                      for tloc in range(4):
                          trow = qc * 4 + tloc
                          lsl = slice(tloc * P, (tloc + 1) * P)
                          for cch in range(2):
                              csl = slice(cch * TCH, (cch + 1) * TCH)
                              y_ps = ypsum.tile([P, TCH], F32, name="y_ps")
                              for pair in range(NPAIR):
                                  nc.tensor.matmul(
                                      y_ps,
                                      lhsT=aTs[pair][:, lsl],
                                      rhs=wp_sb[:, pair, csl],
                                      start=(pair == 0),
                                      stop=(pair == NPAIR - 1),
                                      skip_group_check=True,
                                  )
                              y_sb = ypool.tile([P, TCH], F32, name="y_sb")
                              nc.vector.tensor_tensor(
                                  y_sb, y_ps, bp_sb[:, csl], ALU.add
                              )
                              nc.sync.dma_start(out=y_r[trow, :, csl], in_=y_sb)
    nc.finalize()
    return nc


_CACHE = {}


def _get_nc(stop_phase="C"):
    if stop_phase not in _CACHE:
        _CACHE[stop_phase] = build_nc(stop_phase)
    return _CACHE[stop_phase]


def make_in_maps(x, Wq, bq, Wk, bk, Wv, bv, Wp, bp):
    f = np.float32
    x = np.asarray(x, f)
    in_maps = []
    for core in range(8):
        b, g = core // 2, core % 2
        sl = slice(g * GC, (g + 1) * GC)
        in_maps.append(
            {
                "xT": np.ascontiguousarray(x[b].T),
                "wq": np.ascontiguousarray(np.asarray(Wq, f)[:, sl]),
                "wk": np.ascontiguousarray(np.asarray(Wk, f)[:, sl]),
                "wv": np.ascontiguousarray(np.asarray(Wv, f)[:, sl]),
                "wp": np.ascontiguousarray(np.asarray(Wp, f)[sl, :]),
                "bq": np.ascontiguousarray(np.asarray(bq, f)[sl]),
                "bk": np.ascontiguousarray(np.asarray(bk, f)[sl]),
                "bv": np.ascontiguousarray(np.asarray(bv, f)[sl]),
                "bp": np.asarray(bp, f) if g == 0 else np.zeros(C, f),
                "ones64": np.ones((P, P), f),
            }
        )
    return in_maps


def run(in_maps, stop_phase="C", **kwargs):
    from concourse.bass_utils import run_bass_kernel_spmd

    return run_bass_kernel_spmd(
        _get_nc(stop_phase), in_maps, core_ids=list(range(8)), **kwargs
    )


def kernel(x, Wq, bq, Wk, bk, Wv, bv, Wp, bp):
    in_maps = make_in_maps(x, Wq, bq, Wk, bk, Wv, bv, Wp, bp)
    res = run(in_maps)
    ys = [r["y"] for r in res.results]
    out = np.stack([ys[2 * b] + ys[2 * b + 1] for b in range(B)])
    return out

